# revision 1
# baseline (speedup 1.0000x reference)
"""Trainium2 Bass kernel for AdaptivePhysicallyConstrainedAttention.

Model (see problem reference): top-k-masked dense attention + residual + LayerNorm.
  mask  = top-3 columns of softmax(band_importance) -> additive -inf bias
  q,k,v = x @ W{q,k,v}.T + b        (B=4, L=2048, D=1024, H=16, hd=64)
  attn  = softmax(q k^T / 8 + bias) v ;  out = LN(x + attn @ Wo.T + bo) * gamma + beta

Sharding: 8 cores = (batch 4) x (query-halves 2). Each core computes K/V for its
full batch (duplicated within the pair) and attends its 1024 query rows — no
collectives. Host prep: top-k mask (tiny), weight transposes, bf16 casts, and a
per-core column permutation of x^T so every core's own query rows sit first
(keeps the graph SPMD-uniform).

On-device structure:
  - scores computed transposed (S^T = K Q^T) so the column mask is a
    per-partition activation bias and exp output feeds the AV matmul as lhsT
  - exp runs 1024-wide out of two PSUM banks (ScalarE is the critical engine;
    wide activations amortize its access latency)
  - V stored [k, head, 65] with a ones column -> AV matmul also produces the
    softmax denominator; normalization is a per-partition scale afterwards
  - all projection work is emitted as background chunks popped between
    attention S-steps so ScalarE never starves
  - matmuls in bf16 (fp32 accumulation), everything else fp32
"""

import sys

if "/opt/trn_rl_repo" not in sys.path:
    sys.path.insert(0, "/opt/trn_rl_repo")

from collections import deque

import numpy as np
import ml_dtypes

import concourse.bass as bass  # noqa: F401  (registers engines)
import concourse.tile as tile
from concourse import bacc, mybir
from concourse.bass_utils import run_bass_kernel_spmd
from concourse.masks import make_identity

BF16 = mybir.dt.bfloat16
FP8 = mybir.dt.float8e4
F32 = mybir.dt.float32
AF = mybir.ActivationFunctionType
OP = mybir.AluOpType

B, L, D, H, HD = 4, 2048, 1024, 16, 64
LQ = L // 2  # query rows per core
P = 128
NCORES = 8
TOPK = 3
SCALE = 1.0 / 8.0
MASK_BIAS = -10000.0
LN_EPS = 1e-5

NIT = D // P        # 8   contraction tiles over D
NOT = D // P        # 8   output tiles over D
NKT = L // P        # 16  key tiles
NQS = LQ // P       # 8   query subtiles
NLT = LQ // P       # 8   own-row tiles


def build_nc():
    nc = bacc.Bacc(None, target_bir_lowering=False, debug=False)

    xT = nc.declare_dram_parameter("xT", [D, L], BF16, isOutput=False)
    xT8 = nc.declare_dram_parameter("xT8", [D, L], FP8, isOutput=False)
    xres = nc.declare_dram_parameter("xres", [LQ, D], F32, isOutput=False)
    wqT = nc.declare_dram_parameter("wqT", [D, D], FP8, isOutput=False)
    wkT = nc.declare_dram_parameter("wkT", [D, D], FP8, isOutput=False)
    wvT = nc.declare_dram_parameter("wvT", [D, D], BF16, isOutput=False)
    woT = nc.declare_dram_parameter("woT", [D, D], BF16, isOutput=False)
    # packed small consts: cols 0:16 bias_k, 16:24 bq, 24:32 bk
    cpack = nc.declare_dram_parameter("cpack", [P, 32], F32, isOutput=False)
    bvb = nc.declare_dram_parameter("bvb", [P, D], BF16, isOutput=False)
    # packed gamma/beta broadcast: cols 0:D gamma, D:2D beta
    gbeta = nc.declare_dram_parameter("gbeta", [P, 2 * D], F32, isOutput=False)
    out = nc.declare_dram_parameter("out", [LQ, D], F32, isOutput=True)

    with tile.TileContext(nc) as tc:
        with (
            tc.tile_pool(name="const", bufs=1) as constp,
            tc.tile_pool(name="big", bufs=1) as bigp,
            tc.tile_pool(name="wstream", bufs=2) as wsp,
            tc.tile_pool(name="ps", bufs=2, space="PSUM") as psp,
            tc.tile_pool(name="shps", bufs=2, space="PSUM") as shpsp,
            tc.tile_pool(name="ctxps", bufs=2, space="PSUM") as ctxpsp,
            tc.tile_pool(name="pt", bufs=27) as ptp,
            tc.tile_pool(name="qkstr", bufs=1) as qkp,
            tc.tile_pool(name="small", bufs=4) as smallp,
            tc.tile_pool(name="io", bufs=2) as iop,
        ):
            # ---- resident tensors; DMAs emitted in startup-priority order ----
            xT_sb = bigp.tile([P, NIT, L], BF16, tag="xT")
            xT8_sb = bigp.tile([P, NIT, L], FP8, tag="xT8")
            nc.sync.dma_start(
                out=xT8_sb[:, :, 0:512],
                in_=xT8[:, 0:512].rearrange("(t p) l -> p t l", p=P),
            )
            for th in range(2):
                nc.sync.dma_start(
                    out=xT_sb[:, 4 * th : 4 * th + 4, 0:512],
                    in_=xT[512 * th : 512 * (th + 1), 0:512].rearrange(
                        "(t p) l -> p t l", p=P
                    ),
                )

            def qk_dma(ot, w_dram):
                wt = wsp.tile([P, NIT, P], FP8, tag="wqk")
                nc.sync.dma_start(
                    out=wt[:],
                    in_=w_dram[:, ot * P : (ot + 1) * P].rearrange(
                        "(t p) o -> p t o", p=P
                    ),
                )
                return wt

            wq0 = qk_dma(0, wqT)
            wk0 = qk_dma(0, wkT)
            cp_sb = constp.tile([P, 32], F32, tag="cpack")
            nc.sync.dma_start(out=cp_sb[:], in_=cpack[:, :])
            biask_sb = cp_sb[:, 0:16]
            bq_sb = cp_sb[:, 16:24]
            bk_sb = cp_sb[:, 24:32]
            for lch in range(1, 4):
                nc.sync.dma_start(
                    out=xT8_sb[:, :, lch * 512 : (lch + 1) * 512],
                    in_=xT8[:, lch * 512 : (lch + 1) * 512].rearrange(
                        "(t p) l -> p t l", p=P
                    ),
                )
                nc.sync.dma_start(
                    out=xT_sb[:, :, lch * 512 : (lch + 1) * 512],
                    in_=xT[:, lch * 512 : (lch + 1) * 512].rearrange(
                        "(t p) l -> p t l", p=P
                    ),
                )

            def v_dma(og):
                wt = wsp.tile([P, NIT, 512], BF16, tag="wv")
                nc.sync.dma_start(
                    out=wt[:],
                    in_=wvT[:, og * 512 : (og + 1) * 512].rearrange(
                        "(t p) o -> p t o", p=P
                    ),
                )
                return wt

            wv0 = v_dma(0)
            bvb_sb = constp.tile([P, D], BF16, tag="bvb")
            nc.sync.dma_start(out=bvb_sb[:], in_=bvb[:, :])

            eps_sb = constp.tile([P, 1], F32, tag="eps")
            nc.vector.memset(eps_sb[:], LN_EPS)
            warm = constp.tile([P, 1], F32, tag="warm")
            nc.scalar.activation(out=warm[:], in_=eps_sb[:], func=AF.Exp)
            ident = constp.tile([P, P], BF16, tag="ident")
            make_identity(nc, ident[:])

            v_sb = bigp.tile([P, NKT, H, HD + 1], BF16, tag="v")
            ctxT_sb = bigp.tile([P, NIT, LQ], BF16, tag="ctxT")

            # ones column of the augmented V (softmax denominator trick)
            nc.vector.memset(v_sb[:, :, :, HD : HD + 1], 1.0)

            # ---- projection chunk emitters (each ~1.7us of PE work) ----
            def qk_chunk(wt, ot, bias_sb, dst_tile, lc):
                ps = shpsp.tile([P, 512], F32, tag="shps")
                for i in range(NIT // 2):
                    nc.tensor.matmul(
                        ps[:],
                        wt[:, 2 * i : 2 * i + 2, :],
                        xT8_sb[:, 2 * i : 2 * i + 2, lc * 512 : (lc + 1) * 512],
                        start=(i == 0),
                        stop=(i == NIT // 2 - 1),
                        perf_mode=mybir.MatmulPerfMode.DoubleRow,
                    )
                nc.vector.tensor_scalar(
                    out=dst_tile[:, 0, lc * 512 : (lc + 1) * 512],
                    in0=ps[:],
                    scalar1=bias_sb[:, ot : ot + 1],
                    scalar2=None,
                    op0=OP.add,
                )

            # ping-pong Q/K stream tiles with a zeroed second Ko slice:
            # DoubleRow contracts (p, i) pairs; slice i=1 stays zero so the
            # matmul adds nothing while streaming at 0.5 cycles/row
            qk8 = []
            for pp in range(2):
                qt8 = qkp.tile([P, 2, LQ], FP8, tag=f"q8{pp}")
                kt8 = qkp.tile([P, 2, L], FP8, tag=f"k8{pp}")
                nc.gpsimd.memset(qt8[:, 1, :], 0.0)
                nc.gpsimd.memset(kt8[:, 1, :], 0.0)
                qk8.append((qt8, kt8))

            qk_tiles = {}

            def alloc_qk(ot):
                qt8, kt8 = qk8[ot % 2]
                qk_tiles[ot] = (qt8, kt8)
                return qt8, kt8

            def v_chunk(wt, og, lt):
                ps = shpsp.tile([P, 512], F32, tag="shps")
                for it in range(NIT):
                    nc.tensor.matmul(
                        ps[:],
                        xT_sb[:, it, lt * P : (lt + 1) * P],
                        wt[:, it, :],
                        start=(it == 0),
                        stop=(it == NIT - 1),
                    )
                nc.vector.tensor_tensor(
                    out=v_sb[:, lt, 8 * og : 8 * og + 8, 0:HD],
                    in0=ps[:].rearrange("p (h d) -> p h d", h=8),
                    in1=bvb_sb[:, og * 512 : (og + 1) * 512].rearrange(
                        "p (h d) -> p h d", h=8
                    ),
                    op=OP.add,
                )

            bg_urgent = deque()
            bg = deque()
            _step = [0]

            def bg_pop(force=False):
                # urgent chunks drain greedily; paced chunks every other slot
                if bg_urgent:
                    bg_urgent.popleft()()
                    return
                _step[0] += 1
                if bg and (force or _step[0] % 8 == 0):
                    bg.popleft()()

            # ---- attention: one 1024-wide q chunk per head ----
            # software-pipelined across heads: head h's AV/normalize phase is
            # interleaved with head h+1's S/exp phase so PE always has S work
            # while DVE normalizes and ACT streams exps.

            def av_phase(h, p_tiles, q0=0, qw=LQ):
                po = (h % 2) * HD
                ot = h // 2
                tr_ps = shpsp.tile([HD, qw], BF16, tag="shps")
                for qs in range(qw // P):
                    ctx_ps = ctxpsp.tile([P, HD + 1], F32, tag="ctx")
                    for kt in range(NKT):
                        nc.tensor.matmul(
                            ctx_ps[:],
                            p_tiles[kt][:, qs * P : (qs + 1) * P],
                            v_sb[:, kt, h, :],
                            start=(kt == 0),
                            stop=(kt == NKT - 1),
                        )
                    den = smallp.tile([P, 1], F32, tag="den")
                    nc.vector.reciprocal(den[:], ctx_ps[:, HD : HD + 1])
                    cn = smallp.tile([P, HD], BF16, tag="cn")
                    nc.vector.tensor_scalar(
                        out=cn[:],
                        in0=ctx_ps[:, 0:HD],
                        scalar1=den[:, 0:1],
                        scalar2=None,
                        op0=OP.mult,
                    )
                    nc.tensor.transpose(
                        tr_ps[:, qs * P : (qs + 1) * P], cn[:], ident[:]
                    )
                    bg_pop()
                    yield
                nc.vector.tensor_copy(
                    out=ctxT_sb[po : po + HD, ot, q0 : q0 + qw], in_=tr_ps[:]
                )

            def run_interleaved(gens_weights):
                """Round-robin generators: (gen, steps_per_turn)."""
                live = [[g, w] for g, w in gens_weights]
                while live:
                    for gw in list(live):
                        g, w = gw
                        for _ in range(w):
                            try:
                                next(g)
                            except StopIteration:
                                live.remove(gw)
                                break

            # ---- startup: eagerly project what head 0 needs first ----
            qt0, ktl0 = alloc_qk(0)
            for lc in range(2):
                qk_chunk(wq0, 0, bq_sb, qt0, lc)
            for lc in range(4):
                qk_chunk(wk0, 0, bk_sb, ktl0, lc)
            for lt in range(4):
                v_chunk(wv0, 0, lt)

            wv1 = [None]

            def queue_group(ot):
                # work queued at group ot, popped during its heads' S-steps
                if ot == 0:
                    for lt in range(4, NKT):
                        bg_urgent.append(lambda lt=lt: v_chunk(wv0, 0, lt))
                if ot < NOT - 1:
                    wtq = qk_dma(ot + 1, wqT)
                    wtk = qk_dma(ot + 1, wkT)
                    qt, ktl = alloc_qk(ot + 1)
                    for lc in range(2):
                        bg.append(
                            lambda wt=wtq, ot=ot, lc=lc, qt=qt: qk_chunk(
                                wt, ot + 1, bq_sb, qt, lc
                            )
                        )
                    for lc in range(4):
                        bg.append(
                            lambda wt=wtk, ot=ot, lc=lc, ktl=ktl: qk_chunk(
                                wt, ot + 1, bk_sb, ktl, lc
                            )
                        )
                if ot == 0:
                    wv1[0] = v_dma(1)
                if 0 <= ot <= 3:
                    # paced: V chunks are the expensive filler now; the group
                    # boundary force-drain still meets av(8)'s emission deadline
                    for lt in range(4 * ot, 4 * ot + 4):
                        bg.append(lambda lt=lt: v_chunk(wv1[0], 1, lt))

            # run the pipeline: S(0); then for h: interleave AV(h-1) with S(h)
            class HeadState:
                pass

            def make_s(h, q0=0, qw=LQ):
                st = HeadState()
                st.tiles = []
                po = (h % 2) * HD
                ot = h // 2

                def gen():
                    qt8, kt8 = qk_tiles[ot]
                    for kt in range(NKT):
                        sps = psp.tile([P, qw], F32, tag="sps")
                        for qh in range(qw // 512):
                            nc.tensor.matmul(
                                sps[:, qh * 512 : (qh + 1) * 512],
                                kt8[po : po + HD, :, kt * P : (kt + 1) * P],
                                qt8[
                                    po : po + HD, :,
                                    q0 + qh * 512 : q0 + (qh + 1) * 512,
                                ],
                                start=True,
                                stop=True,
                                perf_mode=mybir.MatmulPerfMode.DoubleRow,
                            )
                        pt = ptp.tile([P, qw], BF16, tag="pt")
                        nc.scalar.activation(
                            out=pt[:],
                            in_=sps[:],
                            func=AF.Exp,
                            bias=biask_sb[:, kt : kt + 1],
                            scale=SCALE,
                        )
                        st.tiles.append(pt)
                        bg_pop()
                        yield

                st.gen = gen()
                return st

            # ---- output projection + residual + layernorm ----
            def p3_setup():
                gb_sb = bigp.tile([P, 2 * D], F32, tag="xT")  # reuses xT's slot
                nc.sync.dma_start(out=gb_sb[:], in_=gbeta[:, :])
                wo_tiles = []
                for oc in range(2):
                    wt = wsp.tile([P, NIT, 512], BF16, tag="wv")  # reuses wv slots
                    nc.sync.dma_start(
                        out=wt[:],
                        in_=woT[:, oc * 512 : (oc + 1) * 512].rearrange(
                            "(t p) o -> p t o", p=P
                        ),
                    )
                    wo_tiles.append(wt)
                return gb_sb[:, 0:D], gb_sb[:, D : 2 * D], wo_tiles

            def p3_gen(lts, gamb_sb, betb_sb, wo_tiles):
                for lt in lts:
                    xr = iop.tile([P, D], F32, tag="xr")
                    nc.sync.dma_start(
                        out=xr[:], in_=xres[lt * P : (lt + 1) * P, :]
                    )
                    y = iop.tile([P, D], F32, tag="y")
                    for oc in range(2):
                        ps = shpsp.tile([P, 512], F32, tag="shps")
                        for it in range(NIT):
                            nc.tensor.matmul(
                                ps[:],
                                ctxT_sb[:, it, lt * P : (lt + 1) * P],
                                wo_tiles[oc][:, it, :],
                                start=(it == 0),
                                stop=(it == NIT - 1),
                            )
                        nc.vector.tensor_tensor(
                            out=y[:, oc * 512 : (oc + 1) * 512],
                            in0=ps[:],
                            in1=xr[:, oc * 512 : (oc + 1) * 512],
                            op=OP.add,
                        )
                        yield
                    stats = smallp.tile([P, 2, 6], F32, tag="stats")
                    nc.vector.bn_stats(stats[:, 0, :], y[:, 0:512])
                    nc.vector.bn_stats(stats[:, 1, :], y[:, 512:1024])
                    mv = smallp.tile([P, 2], F32, tag="mv")
                    nc.vector.bn_aggr(mv[:], stats[:])
                    std = smallp.tile([P, 1], F32, tag="std")
                    nc.scalar.activation(
                        out=std[:], in_=mv[:, 1:2], func=AF.Sqrt,
                        bias=eps_sb[:, 0:1],
                    )
                    rstd = smallp.tile([P, 1], F32, tag="rstd")
                    nc.vector.reciprocal(rstd[:], std[:])
                    nmr = smallp.tile([P, 1], F32, tag="nmr")  # -mu * rstd
                    nc.vector.tensor_scalar(
                        out=nmr[:],
                        in0=mv[:, 0:1],
                        scalar1=rstd[:, 0:1],
                        scalar2=-1.0,
                        op0=OP.mult,
                        op1=OP.mult,
                    )
                    yn = iop.tile([P, D], F32, tag="xr")
                    # (y - mu) * rstd on the (otherwise idle) scalar engine
                    nc.scalar.activation(
                        out=yn[:],
                        in_=y[:],
                        func=AF.Identity,
                        bias=nmr[:, 0:1],
                        scale=rstd[:, 0:1],
                    )
                    o_sb = iop.tile([P, D], F32, tag="y")
                    # gamma*yn + beta: low half on DVE, high half on gpsimd
                    nc.vector.tensor_tensor(
                        out=o_sb[:, 0:512], in0=yn[:, 0:512],
                        in1=gamb_sb[:, 0:512], op=OP.mult,
                    )
                    nc.vector.tensor_tensor(
                        out=o_sb[:, 0:512], in0=o_sb[:, 0:512],
                        in1=betb_sb[:, 0:512], op=OP.add,
                    )
                    nc.gpsimd.tensor_tensor(
                        out=o_sb[:, 512:D], in0=yn[:, 512:D],
                        in1=gamb_sb[:, 512:D], op=OP.mult,
                    )
                    nc.gpsimd.tensor_tensor(
                        out=o_sb[:, 512:D], in0=o_sb[:, 512:D],
                        in1=betb_sb[:, 512:D], op=OP.add,
                    )
                    nc.sync.dma_start(
                        out=out[lt * P : (lt + 1) * P, 0:512], in_=o_sb[:, 0:512]
                    )
                    nc.sync.dma_start(
                        out=out[lt * P : (lt + 1) * P, 512:D], in_=o_sb[:, 512:D]
                    )
                    yield

            queue_group(0)
            st = make_s(0)
            for _ in st.gen:
                pass
            for h in range(1, H):
                if h % 2 == 0:
                    # deadline: previous group's Q/K chunks must be emitted
                    # before this group's S-phase reads them
                    while bg:
                        bg.popleft()()
                    queue_group(h // 2)
                st_next = make_s(h)
                run_interleaved([(av_phase(h - 1, st.tiles), 1), (st_next.gen, 3)])
                st = st_next
            for _ in av_phase(H - 1, st.tiles):
                pass
            while bg_urgent:
                bg_urgent.popleft()()
            while bg:
                bg.popleft()()
            gamb_sb, betb_sb, wo_tiles = p3_setup()
            for _ in p3_gen(range(NLT), gamb_sb, betb_sb, wo_tiles):
                pass

    nc.compile()
    return nc


def host_prep(inputs):
    """Shard + lay out the full inputs into 8 per-core in_maps."""
    bf16 = ml_dtypes.bfloat16
    x = np.asarray(inputs["x"], dtype=np.float32)
    bi = np.asarray(inputs["band_importance"], dtype=np.float32)[0]
    idx = np.argpartition(-bi, TOPK)[:TOPK]  # top-k of softmax == top-k of logits
    bias_vec = np.zeros(L, np.float32)
    bias_vec[idx] = MASK_BIAS

    fp8 = ml_dtypes.float8_e4m3
    wqTn = np.ascontiguousarray(np.asarray(inputs["Wq"], np.float32).T).astype(fp8)
    wkTn = np.ascontiguousarray(np.asarray(inputs["Wk"], np.float32).T).astype(fp8)
    wvTn = np.ascontiguousarray(np.asarray(inputs["Wv"], np.float32).T).astype(bf16)
    woTn = np.ascontiguousarray(np.asarray(inputs["Wo"], np.float32).T).astype(bf16)
    bq = np.asarray(inputs["bq"], np.float32).reshape(NOT, P).T
    bk = np.asarray(inputs["bk"], np.float32).reshape(NOT, P).T
    bv = np.asarray(inputs["bv"], np.float32)
    bo = np.asarray(inputs["bo"], np.float32)
    gam = np.asarray(inputs["gamma"], np.float32)
    bet = np.asarray(inputs["beta"], np.float32)
    bvb = np.ascontiguousarray(np.broadcast_to(bv, (P, D))).astype(bf16)
    gbeta = np.ascontiguousarray(
        np.concatenate(
            [np.broadcast_to(gam, (P, D)), np.broadcast_to(bet, (P, D))], axis=1
        )
    )

    in_maps = []
    for c in range(NCORES):
        b, hh = c // 2, c % 2
        own = slice(hh * LQ, (hh + 1) * LQ)
        oth = slice((1 - hh) * LQ, (2 - hh) * LQ)
        xTb = x[b].T  # [D, L] view
        xT_cf = np.concatenate([xTb[:, own], xTb[:, oth]], axis=1)
        xT_c = xT_cf.astype(bf16)
        xT8_c = xT_cf.astype(fp8)
        pb = np.concatenate([bias_vec[own], bias_vec[oth]])
        biask_c = pb.reshape(NKT, P).T
        cpack_c = np.ascontiguousarray(
            np.concatenate([biask_c, bq, bk], axis=1), dtype=np.float32
        )
        xres_c = np.ascontiguousarray(x[b, own]) + bo[None, :]
        in_maps.append(
            {
                "xT": xT_c,
                "xT8": xT8_c,
                "xres": xres_c,
                "wqT": wqTn,
                "wkT": wkTn,
                "wvT": wvTn,
                "woT": woTn,
                "cpack": cpack_c,
                "bvb": bvb,
                "gbeta": gbeta,
            }
        )
    return in_maps


def assemble(results):
    out = np.empty((B, L, D), np.float32)
    for c in range(NCORES):
        b, hh = c // 2, c % 2
        out[b, hh * LQ : (hh + 1) * LQ, :] = results[c]["out"]
    return out


_NC_CACHE = None


def kernel(**inputs):
    global _NC_CACHE
    if _NC_CACHE is None:
        _NC_CACHE = build_nc()
    in_maps = host_prep(inputs)
    res = run_bass_kernel_spmd(_NC_CACHE, in_maps, core_ids=list(range(NCORES)))
    return assemble(res.results)



# revision 7
# speedup vs baseline: 1.4247x; 1.4247x over previous
"""Trainium2 Bass kernel for AdaptivePhysicallyConstrainedAttention, v3.

Model: top-k-masked dense attention + residual + LayerNorm (B=4, L=2048,
D=1024, H=16, hd=64). Sharding: 8 cores = (batch 4) x (query-halves 2),
no collectives; each core attends its 1024 query rows in two 512-row
phases (phase-0 output projection + LayerNorm overlaps phase-1
attention).

The kernel is elementwise-exp bound, so exp is split across THREE
engines; scores land in PSUM as bf16 [128 keys, 2, 512 q] kt-pair tiles
(bf16 so a pair is one PSUM bank -> 3 tiles in flight), each exp'd
1024-wide by one of:
  ACT : native Exp -> fp8e4 pair tile (feeds DoubleRow AV)
  DVE : Schraudolph fast-exp: one f32 affine f = A*s + (B + 1.5*2^23);
        the low 16 bits of each f32 are exactly the bf16 of 2^(A's+B'),
        read via a stride-2 bitcast view (no second op)
  Pool: same trick on gpsimd
The top-k mask is applied by zeroing masked KEY ROWS of V (incl. the
ones-column used for the softmax denominator), so exp needs no mask
bias and the fast paths need no clamp; bv/bo are folded into the
residual host-side. AV accumulates all 4 query-subtiles of a head into
one PSUM bank; softmax normalization runs entirely on Pool
(normalize_recip); transposes batch per head-pair into one PSUM bank.
LayerNorm runs as per-row-tile single-engine chains (alternating
DVE/Pool) with a bit-trick rsqrt, keeping ACT exp-only (no activation
table thrash). All projections are fp8 DoubleRow.
"""

import sys

if "/opt/trn_rl_repo" not in sys.path:
    sys.path.insert(0, "/opt/trn_rl_repo")

from collections import deque

import numpy as np
import ml_dtypes

import concourse.bass as bass  # noqa: F401  (registers engines)
import concourse.tile as tile
from concourse import bacc, mybir
from concourse.bass_utils import run_bass_kernel_spmd
from concourse.masks import make_identity

BF16 = mybir.dt.bfloat16
FP8 = mybir.dt.float8e4
FP8E5 = mybir.dt.float8e5
F32 = mybir.dt.float32
I32 = mybir.dt.int32
AF = mybir.ActivationFunctionType
OP = mybir.AluOpType
DR = mybir.MatmulPerfMode.DoubleRow

B, L, D, H, HD = 4, 2048, 1024, 16, 64
LQ = L // 2        # query rows per core
HF = 512           # query rows per phase
P = 128
NCORES = 8
TOPK = 3
SCALE = 1.0 / 8.0
LN_EPS = 1e-5
SHIFT = -2.0       # constant exp shift (cancels in softmax; keeps fp8 happy)

NIT = D // P       # 8  contraction tiles over D
NOT = D // P       # 8  output tiles over D
NKT = L // P       # 16 key tiles
NPAIR = NKT // 2   # 8  kt pairs per (head, phase)
NQS = HF // P      # 4  query subtiles per phase
NLT = LQ // P      # 8  own-row tiles

LOG2E = float(np.log2(np.e))
ANCHOR = 1.5 * 2.0**23
FE_A = 4.0 * LOG2E * SCALE
FE_B = 15.0 * 4.0 - 0.23 + 4.0 * LOG2E * SHIFT + ANCHOR
RSQRT_C = 0x5F3759DF

# per-(phase, head%2) engine assignment of the 8 kt-pair exp tiles
# A=ACT(fp8 native exp), D=DVE(fast-exp), G=Pool/gpsimd(fast-exp).
# Phase 0 is projection/PE-heavy (exp engines have slack); phase 1
# carries the phase-0 LN chains on DVE/Pool, so ACT gets more there.
EXP_PATTERNS = {
    "ph0": [
        ["A", "D", "A", "A", "D", "A", "D", "A"],  # A5 D3
        ["A", "D", "A", "A", "D", "A", "D", "A"],
    ],
    "ph1e": [
        ["A", "D", "A", "A", "D", "A", "D", "A"],
        ["A", "D", "A", "A", "D", "A", "D", "A"],
    ],
    "ph1l": [
        ["A", "D", "A", "A", "D", "A", "D", "A"],
        ["A", "D", "A", "A", "D", "A", "D", "A"],
    ],
}


def build_nc():
    nc = bacc.Bacc(None, target_bir_lowering=False, debug=False)

    xT8 = nc.declare_dram_parameter("xT8", [D, L], FP8, isOutput=False)
    wq8 = nc.declare_dram_parameter("wq8", [D, D], FP8, isOutput=False)
    wk8 = nc.declare_dram_parameter("wk8", [D, D], FP8, isOutput=False)
    wv8 = nc.declare_dram_parameter("wv8", [D, D], FP8, isOutput=False)
    wo8 = nc.declare_dram_parameter("wo8", [D, D], FP8, isOutput=False)
    # packed small consts: cols 0:16 maskcol, 16:24 bq, 24:32 bk
    cpack = nc.declare_dram_parameter("cpack", [P, 32], F32, isOutput=False)
    xres = nc.declare_dram_parameter("xres", [LQ, D], F32, isOutput=False)
    # packed gamma/beta broadcast: cols 0:D gamma, D:2D beta
    gbeta = nc.declare_dram_parameter("gbeta", [P, 2 * D], F32, isOutput=False)
    out = nc.declare_dram_parameter("out", [LQ, D], F32, isOutput=True)

    with tile.TileContext(nc) as tc:
        with (
            tc.tile_pool(name="const", bufs=1) as constp,
            tc.tile_pool(name="big", bufs=1) as bigp,
            tc.tile_pool(name="wstream", bufs=2) as wsp,
            tc.tile_pool(name="ps", bufs=3, space="PSUM") as psp,
            tc.tile_pool(name="aux", bufs=2, space="PSUM") as auxp,
            tc.tile_pool(name="pt8", bufs=13) as pt8p,
            tc.tile_pool(name="pf32", bufs=13) as pfp,
            tc.tile_pool(name="cn", bufs=3) as cnp,
            tc.tile_pool(name="ctxs", bufs=2) as ctxsp,
            tc.tile_pool(name="small", bufs=8) as smallp,
            tc.tile_pool(name="io", bufs=2) as iop,
        ):
            # ---- resident tensors; DMAs emitted in startup-priority order ----
            xT8_sb = bigp.tile([P, NIT, L], FP8, tag="xT8")

            def qk_dma(ot, w_dram):
                wt = wsp.tile([P, NIT, P], FP8, tag="wqk")
                nc.sync.dma_start(
                    out=wt[:],
                    in_=w_dram[:, ot * P : (ot + 1) * P].rearrange(
                        "(t p) o -> p t o", p=P
                    ),
                )
                return wt

            wk0 = qk_dma(0, wk8)
            wq0 = qk_dma(0, wq8)
            cp_sb = constp.tile([P, 32], F32, tag="cpack")
            nc.sync.dma_start(out=cp_sb[:], in_=cpack[:, :])
            maskcol = cp_sb[:, 0:16]
            bq_sb = cp_sb[:, 16:24]
            bk_sb = cp_sb[:, 24:32]
            for lch in range(0, 4):
                nc.sync.dma_start(
                    out=xT8_sb[:, :, lch * 512 : (lch + 1) * 512],
                    in_=xT8[:, lch * 512 : (lch + 1) * 512].rearrange(
                        "(t p) l -> p t l", p=P
                    ),
                )

            def wv_dma(og, w_dram):
                wt = wsp.tile([P, NIT, 512], FP8, tag="wv")
                nc.sync.dma_start(
                    out=wt[:],
                    in_=w_dram[:, og * 512 : (og + 1) * 512].rearrange(
                        "(t p) o -> p t o", p=P
                    ),
                )
                return wt

            wv0 = wv_dma(0, wv8)

            shift_sb = constp.tile([P, 1], F32, tag="shift")
            nc.vector.memset(shift_sb[:], SHIFT)
            warm = constp.tile([P, 1], F32, tag="warm")
            nc.scalar.activation(out=warm[:], in_=shift_sb[:], func=AF.Exp)
            ident = constp.tile([P, P], BF16, tag="ident")
            make_identity(nc, ident[:])

            # K/Q streams: block 8 is a shared zero block, reached per-ot by
            # step-slicing t[:, ot::(8-ot), :] -> blocks {ot, 8}; the zero
            # side makes the DoubleRow second slice contribute nothing.
            kt8 = bigp.tile([P, NOT + 1, L], FP8, tag="kt8")
            qt8 = bigp.tile([P, NOT + 1, LQ], FP8, tag="qt8")
            nc.vector.memset(qt8[:, NOT, :], 0.0)
            nc.vector.memset(kt8[:, NOT, :], 0.0)

            v8 = bigp.tile([P, NKT, H, HD + 1], FP8E5, tag="v8")
            ctxT = bigp.tile([P, NIT, LQ], FP8, tag="ctxT")

            # ones columns get the 0/1 key mask directly
            for kt in range(NKT):
                nc.vector.tensor_copy(
                    out=v8[:, kt, :, HD],
                    in_=maskcol[:, kt : kt + 1].to_broadcast([P, H]),
                )

            # ---- projection chunk emitters ----
            def qk_chunk(wt, ot, bias_sb, dst, col0, on_act=True):
                ps = auxp.tile([P, 512], F32, tag="aux", name="ps")
                for i in range(NIT // 2):
                    nc.tensor.matmul(
                        ps[:],
                        wt[:, 2 * i : 2 * i + 2, :],
                        xT8_sb[:, 2 * i : 2 * i + 2, col0 : col0 + 512],
                        start=(i == 0),
                        stop=(i == NIT // 2 - 1),
                        perf_mode=DR,
                    )
                if on_act:
                    nc.scalar.activation(
                        out=dst[:, ot, col0 : col0 + 512],
                        in_=ps[:],
                        func=AF.Identity,
                        bias=bias_sb[:, ot : ot + 1],
                        scale=1.0,
                    )
                else:
                    nc.vector.tensor_scalar(
                        out=dst[:, ot, col0 : col0 + 512],
                        in0=ps[:],
                        scalar1=bias_sb[:, ot : ot + 1],
                        scalar2=None,
                        op0=OP.add,
                    )

            def v_chunk(wt, og, lt):
                ps = auxp.tile([P, 512], F32, tag="aux", name="ps")
                for i in range(NIT // 2):
                    nc.tensor.matmul(
                        ps[:],
                        xT8_sb[:, 2 * i : 2 * i + 2, lt * P : (lt + 1) * P],
                        wt[:, 2 * i : 2 * i + 2, :],
                        start=(i == 0),
                        stop=(i == NIT // 2 - 1),
                        perf_mode=DR,
                    )
                nc.vector.tensor_scalar(
                    out=v8[:, lt, 8 * og : 8 * og + 8, 0:HD],
                    in0=ps[:].rearrange("p (h d) -> p h d", h=8),
                    scalar1=maskcol[:, lt : lt + 1],
                    scalar2=None,
                    op0=OP.mult,
                )

            bg = deque()

            def bg_pop():
                if bg:
                    bg.popleft()()

            # ---- attention blocks ----
            ptiles = {}
            cn_tiles = {}

            def s_block(h, q0, phase):
                """S^T (f32 psum kt-pairs) + 3-way exp for head h."""
                po = (h % 2) * HD
                ot = h // 2
                tiles = []
                ptiles[(phase * H + h) % 3] = tiles

                def gen():
                    for b in range(NPAIR):
                        sps = psp.tile([P, 2, HF], F32, tag="sps")
                        for j in range(2):
                            kt = 2 * b + j
                            nc.tensor.matmul(
                                sps[:, j, :],
                                kt8[po : po + HD, ot :: (NOT - ot), kt * P : (kt + 1) * P],
                                qt8[po : po + HD, ot :: (NOT - ot), q0 : q0 + HF],
                                start=True,
                                stop=True,
                                perf_mode=DR,
                            )
                        pk = "ph0" if phase == 0 else ("ph1e" if h < 6 else "ph1l")
                        kind = EXP_PATTERNS[pk][h % 2][b]
                        if kind == "A":
                            pt = pt8p.tile([P, 2, HF], FP8E5, tag="pt8")
                            nc.scalar.activation(
                                out=pt[:],
                                in_=sps[:],
                                func=AF.Exp,
                                bias=shift_sb[:, 0:1],
                                scale=SCALE,
                            )
                            tiles.append(("8", pt))
                        else:
                            pf = pfp.tile([P, 2, HF], F32, tag="pf32")
                            nc.vector.tensor_scalar(
                                out=pf[:],
                                in0=sps[:],
                                scalar1=FE_A,
                                scalar2=FE_B,
                                op0=OP.mult,
                                op1=OP.add,
                            )
                            tiles.append(("16", pf))
                        bg_pop()
                        yield

                return gen()

            def av_block(h, q0, phase):
                """AV into one per-head psum bank + Pool-normalize + (odd h)
                batched transposes into ctxT."""
                tiles = ptiles[(phase * H + h) % 3]
                ot = h // 2

                def gen():
                    ctx = auxp.tile([P, NQS, P], F32, tag="aux", name="ctx")
                    for qs in range(NQS):
                        for b, (kind, pt) in enumerate(tiles):
                            if kind == "8":
                                lhsT = pt[:, :, qs * P : (qs + 1) * P]
                            else:
                                lhsT = pt[:].bitcast(FP8E5)[
                                    :, :, qs * 4 * P : (qs + 1) * 4 * P : 4
                                ]
                            nc.tensor.matmul(
                                ctx[:, qs, 0 : HD + 1],
                                lhsT,
                                v8[:, 2 * b : 2 * b + 2, h, :],
                                start=(b == 0),
                                stop=(b == NPAIR - 1),
                                perf_mode=DR,
                            )
                        bg_pop()
                        yield
                    # normalize all 4 qs on Pool (no cross-engine chain)
                    ctxs = ctxsp.tile([P, NQS, HD + 1], F32, tag="ctxs")
                    nc.scalar.activation(
                        out=ctxs[:], in_=ctx[:, :, 0 : HD + 1],
                        func=AF.Identity, bias=0.0, scale=1.0,
                    )
                    for qs in range(NQS):
                        if h % 2 == 0:
                            cn_tiles[qs] = cnp.tile(
                                [P, 2, HD], BF16, tag=f"cn{qs}", name=f"cnp{qs}"
                            )
                        nc.gpsimd.normalize_recip(
                            cn_tiles[qs][:, h % 2, :],
                            ctxs[:, qs, 0:HD],
                            ctxs[:, qs, HD : HD + 1],
                        )
                    if h % 2 == 1:
                        tr = auxp.tile([P, NQS, P], BF16, tag="aux", name="tr")
                        for qs in range(NQS):
                            nc.tensor.transpose(
                                tr[:, qs, :],
                                cn_tiles[qs][:].rearrange("p a b -> p (a b)"),
                                ident[:],
                            )
                        nc.vector.tensor_copy(
                            out=ctxT[:, ot, q0 : q0 + HF],
                            in_=tr[:].rearrange("p a b -> p (a b)"),
                        )
                    bg_pop()
                    yield

                return gen()

            def run_interleaved(gens_weights):
                live = [[g, w] for g, w in gens_weights]
                while live:
                    for gw in list(live):
                        g, w = gw
                        for _ in range(w):
                            try:
                                next(g)
                            except StopIteration:
                                live.remove(gw)
                                break

            # ---- output projection + residual + layernorm ----
            def p3_setup():
                gb_sb = bigp.tile([P, 2 * D], F32, tag="gb")
                nc.sync.dma_start(out=gb_sb[:], in_=gbeta[:, :])
                wo_tiles = [wv_dma(0, wo8), wv_dma(1, wo8)]
                return gb_sb[:, 0:D], gb_sb[:, D : 2 * D], wo_tiles

            def rsqrt_ln(eng, ve, lt):
                """rstd = 1/sqrt(ve) via shift-seed + 2 Newton steps.
                Bit ops (shift/not on int32) are DVE-only on real HW."""
                eng = nc.vector
                t1 = smallp.tile([P, 1], I32, tag="rst1", name=f"rst1_{lt}")
                eng.tensor_scalar(
                    out=t1[:], in0=ve.bitcast(I32), scalar1=1, scalar2=None,
                    op0=OP.logical_shift_right,
                )
                eng.tensor_scalar(
                    out=t1[:], in0=t1[:], scalar1=-RSQRT_C - 1, scalar2=None,
                    op0=OP.add,
                )
                y0i = smallp.tile([P, 1], I32, tag="rsy0", name=f"rsy0_{lt}")
                eng.tensor_tensor(out=y0i[:], in0=t1[:], in1=t1[:], op=OP.bitwise_not)
                y0f = smallp.tile([P, 1], F32, tag="rsy0f", name=f"rsy0f_{lt}")
                eng.tensor_copy(out=y0f[:], in_=y0i[:].bitcast(F32))
                y = y0f[:]
                for it in range(2):
                    a = smallp.tile([P, 1], F32, tag=f"rsa{it}", name=f"rsa{it}_{lt}")
                    eng.tensor_tensor(out=a[:], in0=ve, in1=y, op=OP.mult)
                    eng.tensor_tensor(out=a[:], in0=a[:], in1=y, op=OP.mult)
                    eng.tensor_scalar(
                        out=a[:], in0=a[:], scalar1=-0.5, scalar2=1.5,
                        op0=OP.mult, op1=OP.add,
                    )
                    yn = smallp.tile([P, 1], F32, tag=f"rsy{it}", name=f"rsy{it}_{lt}")
                    eng.tensor_tensor(out=yn[:], in0=a[:], in1=y, op=OP.mult)
                    y = yn[:]
                return y

            def p3_gen(lts, gamb_sb, betb_sb, wo_tiles, split=False):
                """Per row-tile: O-proj (PE) then the LN chain. split=False:
                one engine per row-tile (alternating DVE/Pool) so the chain
                never blocks another engine's exp stream. split=True (tail,
                no exp contention): spread across DVE+Pool+ACT."""
                for lt in lts:
                    eng = nc.vector if lt % 2 == 0 else nc.gpsimd
                    xr = iop.tile([P, D], F32, tag="xr")
                    nc.sync.dma_start(out=xr[:], in_=xres[lt * P : (lt + 1) * P, :])
                    y = iop.tile([P, D], F32, tag="y")
                    sums = smallp.tile([P, 4], F32, tag="sums", name=f"sums_{lt}")
                    sq = iop.tile([P, 512], F32, tag="sq", bufs=1, name=f"sq_{lt}")
                    for oc in range(2):
                        ps = auxp.tile([P, 512], F32, tag="aux", name="ps")
                        for i in range(NIT // 2):
                            nc.tensor.matmul(
                                ps[:],
                                ctxT[:, 2 * i : 2 * i + 2, lt * P : (lt + 1) * P],
                                wo_tiles[oc][:, 2 * i : 2 * i + 2, :],
                                start=(i == 0),
                                stop=(i == NIT // 2 - 1),
                                perf_mode=DR,
                            )
                        yh = y[:, oc * 512 : (oc + 1) * 512]
                        nc.vector.scalar_tensor_tensor(
                            out=yh,
                            in0=ps[:],
                            scalar=1.0,
                            in1=xr[:, oc * 512 : (oc + 1) * 512],
                            op0=OP.mult,
                            op1=OP.add,
                            accum_out=sums[:, oc : oc + 1],
                        )
                        yield
                        if split:
                            nc.scalar.activation(
                                out=sq[:],
                                in_=yh,
                                func=AF.Square,
                                accum_out=sums[:, 2 + oc : 3 + oc],
                            )
                        else:
                            nc.vector.scalar_tensor_tensor(
                                out=sq[:],
                                in0=yh,
                                scalar=1.0,
                                in1=yh,
                                op0=OP.mult,
                                op1=OP.mult,
                                accum_out=sums[:, 2 + oc : 3 + oc],
                            )
                        yield
                    mu = smallp.tile([P, 1], F32, tag="mu", name=f"mu_{lt}")
                    eng.tensor_tensor(
                        out=mu[:], in0=sums[:, 0:1], in1=sums[:, 1:2], op=OP.add
                    )
                    eng.tensor_scalar(
                        out=mu[:], in0=mu[:], scalar1=1.0 / D, scalar2=None,
                        op0=OP.mult,
                    )
                    s2 = smallp.tile([P, 1], F32, tag="s2", name=f"s2_{lt}")
                    eng.tensor_tensor(
                        out=s2[:], in0=sums[:, 2:3], in1=sums[:, 3:4], op=OP.add
                    )
                    musq = smallp.tile([P, 1], F32, tag="musq", name=f"musq_{lt}")
                    eng.tensor_tensor(out=musq[:], in0=mu[:], in1=mu[:], op=OP.mult)
                    ve = smallp.tile([P, 1], F32, tag="ve", name=f"ve_{lt}")
                    eng.tensor_scalar(
                        out=ve[:], in0=s2[:], scalar1=1.0 / D, scalar2=LN_EPS,
                        op0=OP.mult, op1=OP.add,
                    )
                    eng.tensor_tensor(out=ve[:], in0=ve[:], in1=musq[:], op=OP.subtract)
                    rstd = rsqrt_ln(eng, ve[:], lt)
                    nmr = smallp.tile([P, 1], F32, tag="nmr", name=f"nmr_{lt}")
                    eng.tensor_scalar(
                        out=nmr[:], in0=mu[:], scalar1=rstd, scalar2=-1.0,
                        op0=OP.mult, op1=OP.mult,
                    )
                    yn = iop.tile([P, D], F32, tag="xr")
                    if split:
                        nc.scalar.activation(
                            out=yn[:], in_=y[:], func=AF.Identity,
                            bias=nmr[:, 0:1], scale=rstd,
                        )
                    else:
                        eng.tensor_scalar(
                            out=yn[:], in0=y[:], scalar1=rstd, scalar2=nmr[:, 0:1],
                            op0=OP.mult, op1=OP.add,
                        )
                    yield
                    o_sb = iop.tile([P, D], F32, tag="y")
                    if split:
                        for e2, sl in ((nc.vector, slice(0, 512)), (nc.gpsimd, slice(512, D))):
                            e2.tensor_tensor(
                                out=o_sb[:, sl], in0=yn[:, sl], in1=gamb_sb[:, sl], op=OP.mult
                            )
                            e2.tensor_tensor(
                                out=o_sb[:, sl], in0=o_sb[:, sl], in1=betb_sb[:, sl], op=OP.add
                            )
                    else:
                        eng.tensor_tensor(
                            out=o_sb[:], in0=yn[:], in1=gamb_sb[:, 0:D], op=OP.mult
                        )
                        yield
                        eng.tensor_tensor(
                            out=o_sb[:], in0=o_sb[:], in1=betb_sb[:, 0:D], op=OP.add
                        )
                    nc.sync.dma_start(
                        out=out[lt * P : (lt + 1) * P, 0:512], in_=o_sb[:, 0:512]
                    )
                    nc.sync.dma_start(
                        out=out[lt * P : (lt + 1) * P, 512:D], in_=o_sb[:, 512:D]
                    )
                    yield

            # ---- schedule ----
            for kc in range(2):
                qk_chunk(wk0, 0, bk_sb, kt8, kc * 512)
            qk_chunk(wq0, 0, bq_sb, qt8, 0, on_act=False)
            for kc in range(2, 4):
                bg.append(lambda kc=kc: qk_chunk(wk0, 0, bk_sb, kt8, kc * 512))

            wv1 = [None]

            def queue_group(ot, phase):
                if phase == 0:
                    if ot == 0:
                        for lt in range(NKT):
                            bg.append(lambda lt=lt: v_chunk(wv0, 0, lt))
                    if ot == 1:
                        wv1[0] = wv_dma(1, wv8)
                    if 1 <= ot <= 4:
                        for lt in range(4 * (ot - 1), 4 * ot):
                            bg.append(lambda lt=lt: v_chunk(wv1[0], 1, lt))
                    if ot < NOT - 1:
                        wtq = qk_dma(ot + 1, wq8)
                        wtk = qk_dma(ot + 1, wk8)
                        for kc in range(4):
                            bg.append(
                                lambda wt=wtk, ot=ot, kc=kc: qk_chunk(
                                    wt, ot + 1, bk_sb, kt8, kc * 512
                                )
                            )
                        bg.append(
                            lambda wt=wtq, ot=ot: qk_chunk(wt, ot + 1, bq_sb, qt8, 0, on_act=False)
                        )
                else:
                    # phase-1 query projections for the second 512 rows
                    if ot < NOT - 1:
                        wtq = qk_dma(ot + 1, wq8)
                        bg.append(
                            lambda wt=wtq, ot=ot: qk_chunk(wt, ot + 1, bq_sb, qt8, HF, on_act=False)
                        )

            p3A = [None]

            def p3_bg():
                if p3A[0] is not None:
                    next(p3A[0], None)

            # flat schedule over 32 (phase, head) blocks with lag-2 AV and
            # cross-phase interleaving at the boundary
            def blk(i):
                return (i // H, i % H, (i // H) * HF)

            queue_group(0, 0)
            for _ in s_block(0, 0, 0):
                bg_pop()
            for _ in s_block(1, 0, 0):
                bg_pop()
            # everything AV(0) needs (all V og0) must be emitted first
            while bg:
                bg.popleft()()
            for i in range(2, 2 * H):
                phase, h, q0 = blk(i)
                if h % 2 == 0:
                    if phase == 0 and h > 0:
                        queue_group(h // 2, 0)
                    if phase == 0 and h == H - 2:
                        # phase-1 head-0 queries, projected just in time
                        wq0b = qk_dma(0, wq8)
                        bg.append(lambda: qk_chunk(wq0b, 0, bq_sb, qt8, HF, on_act=False))
                    if phase == 1:
                        queue_group(h // 2, 1)
                    if phase == 1 and h == 6:
                        gamb, betb, wo_tiles = p3_setup()
                        p3A[0] = p3_gen(range(4), gamb, betb, wo_tiles)
                    if phase == 1 and h >= 6:
                        for _ in range(7):
                            bg.append(p3_bg)
                pp, hp, qp = blk(i - 2)
                run_interleaved(
                    [(av_block(hp, qp, pp), 1), (s_block(h, q0, phase), 2)]
                )
            for i in (2 * H - 2, 2 * H - 1):
                pp, hp, qp = blk(i)
                for _ in av_block(hp, qp, pp):
                    bg_pop()
            while bg:
                bg.popleft()()

            tail_gens = [(p3_gen(range(4, NLT), gamb, betb, wo_tiles, split=True), 1)]
            if p3A[0] is not None:
                tail_gens.append((p3A[0], 1))
            run_interleaved(tail_gens)

    nc.compile()
    return nc


def host_prep(inputs):
    """Shard + lay out the full inputs into 8 per-core in_maps."""
    fp8 = ml_dtypes.float8_e4m3
    x = np.asarray(inputs["x"], dtype=np.float32)
    bi = np.asarray(inputs["band_importance"], dtype=np.float32)[0]
    idx = np.argpartition(-bi, TOPK)[:TOPK]  # top-k of softmax == top-k of logits
    keep = np.ones(L, np.float32)
    keep[idx] = 0.0

    wq8n = np.ascontiguousarray(np.asarray(inputs["Wq"], np.float32).T).astype(fp8)
    wk8n = np.ascontiguousarray(np.asarray(inputs["Wk"], np.float32).T).astype(fp8)
    wv8n = np.ascontiguousarray(np.asarray(inputs["Wv"], np.float32).T).astype(fp8)
    wo8n = np.ascontiguousarray(np.asarray(inputs["Wo"], np.float32).T).astype(fp8)
    bq = np.asarray(inputs["bq"], np.float32).reshape(NOT, P).T
    bk = np.asarray(inputs["bk"], np.float32).reshape(NOT, P).T
    bv = np.asarray(inputs["bv"], np.float32)
    bo = np.asarray(inputs["bo"], np.float32)
    gam = np.asarray(inputs["gamma"], np.float32)
    bet = np.asarray(inputs["beta"], np.float32)
    res_bias = bo + bv @ np.asarray(inputs["Wo"], np.float32).T
    gbeta = np.ascontiguousarray(
        np.concatenate(
            [np.broadcast_to(gam, (P, D)), np.broadcast_to(bet, (P, D))], axis=1
        )
    )

    in_maps = []
    for c in range(NCORES):
        b, hh = c // 2, c % 2
        own = slice(hh * LQ, (hh + 1) * LQ)
        oth = slice((1 - hh) * LQ, (2 - hh) * LQ)
        xTb = x[b].T
        xT_cf = np.concatenate([xTb[:, own], xTb[:, oth]], axis=1)
        xT8_c = xT_cf.astype(fp8)
        pk = np.concatenate([keep[own], keep[oth]])
        maskcol_c = pk.reshape(NKT, P).T
        cpack_c = np.ascontiguousarray(
            np.concatenate([maskcol_c, bq, bk], axis=1), dtype=np.float32
        )
        xres_c = np.ascontiguousarray(x[b, own]) + res_bias[None, :]
        in_maps.append(
            {
                "xT8": xT8_c,
                "xres": xres_c,
                "wq8": wq8n,
                "wk8": wk8n,
                "wv8": wv8n,
                "wo8": wo8n,
                "cpack": cpack_c,
                "gbeta": gbeta,
            }
        )
    return in_maps


def assemble(results):
    out = np.empty((B, L, D), np.float32)
    for c in range(NCORES):
        b, hh = c // 2, c % 2
        out[b, hh * LQ : (hh + 1) * LQ, :] = results[c]["out"]
    return out


_NC_CACHE = None


def kernel(**inputs):
    global _NC_CACHE
    if _NC_CACHE is None:
        _NC_CACHE = build_nc()
    in_maps = host_prep(inputs)
    res = run_bass_kernel_spmd(_NC_CACHE, in_maps, core_ids=list(range(NCORES)))
    return assemble(res.results)
